# revision 17
# baseline (speedup 1.0000x reference)
"""Trainium2 Bass kernel for a 16-head linear-attention ("ALU") transformer block.

Reference computation (per row r of x, flattened over [B, N]):
    q  = x @ Wq.T                     # [R, 2048] -> 16 heads x 128
    g  = silu(x @ Wg.T)               # [R, 2048]
    e_h = silu(q_h @ (s*k_h).T)       # [R, 4096]   s = sqrt(128)
    o_h = e_h @ (s*v_h)               # [R, 128]
    out = (concat_h(o_h) * g) @ Wout.T

Strategy: pure data-parallel over the 4096 rows (512 rows/core, 8 cores, no
collectives). Weights are replicated; all operands are pre-transposed and cast
to bf16 on the host so every DMA is contiguous and every matmul contracts on
the partition axis. fp32 accumulation in PSUM throughout.

Schedule notes:
- PE warm-up matmuls run before any data lands (HAM ramp + early occupancy).
- Startup DMAs are issued in bandwidth order: x first (it paces Q0), head-0
  k/v split so attention can start as soon as q0 is ready, wq1/wg0 between
  the k/v halves.
- The gate accumulators live in the eps PSUM pool so the acc pool is a clean
  qps/ops alternation — head h's AV matmuls never wait on Q(h+1) completing,
  and gate(h+1) never waits on og-mul(h) through buffer reuse.
- Head 15 has no Q/gate fillers, so the first out-projection tile's matmuls
  are interleaved into its silu-paced group loop, and outproj (0,1) runs on
  an eps-pool psum to hide the og[15] silu->mul tail chain.
"""

import sys
import types

import numpy as np
import ml_dtypes

import concourse.bass as bass
import concourse.mybir as mybir
import concourse.tile as tile
from concourse import bacc
from concourse.bass_utils import run_bass_kernel_spmd

BF16 = mybir.dt.bfloat16
F32 = mybir.dt.float32

P = 128          # partitions / head dim
H = 16           # heads
CE = 16          # contraction chunks over E=2048
R = 512          # rows per core
MI = 32          # m-chunks per head (M=4096)
NCORES = 8
E = 2048
M = 4096
SCALE = float(np.float32(P) ** 0.5)

NGROUPS = 16     # energy groups per head (2 m-chunks each)
N_DUMMY = 5      # PE warm-up matmuls
KV_SPLIT = 12    # head-0 k/v first-DMA chunk count


def install_ntff_hook():
    """Install the axon NTFF profiling hook that the stub `antenv` lacks."""
    import antenv

    if "antenv.axon_hooks" in sys.modules:
        return
    try:
        from trn_agent_boot.trn_boot import _ntff_profile_via_ctypes

        hook = _ntff_profile_via_ctypes("/opt/axon/libaxon_pjrt.so")
    except Exception:
        hook = None
    mod = types.ModuleType("antenv.axon_hooks")
    mod.get_axon_ntff_profile_hook = lambda: hook
    mod.set_axon_ntff_profile_hook = lambda h: None
    sys.modules["antenv.axon_hooks"] = mod
    antenv.axon_hooks = mod


def build_nc():
    nc = bacc.Bacc("TRN2", target_bir_lowering=False, debug=False, num_devices=NCORES)

    xt_d = nc.dram_tensor("xt", [P, CE, R], BF16, kind="ExternalInput").ap()
    wqt_d = nc.dram_tensor("wqt", [H, P, CE, P], BF16, kind="ExternalInput").ap()
    wgt_d = nc.dram_tensor("wgt", [H, P, CE, P], BF16, kind="ExternalInput").ap()
    kt_d = nc.dram_tensor("kt", [H, P, MI, P], BF16, kind="ExternalInput").ap()
    v_d = nc.dram_tensor("v", [H, P, MI, P], BF16, kind="ExternalInput").ap()
    wout_d = nc.dram_tensor("woutt", [4, P, CE, 512], BF16, kind="ExternalInput").ap()
    out_d = nc.dram_tensor("out", [R, E], F32, kind="ExternalOutput").ap()

    with tile.TileContext(nc) as tc:
        with (
            tc.tile_pool(name="const", bufs=1) as const,
            tc.tile_pool(name="wqs", bufs=2) as wqp,
            tc.tile_pool(name="wgs", bufs=2) as wgp,
            tc.tile_pool(name="kv", bufs=2) as kv,
            tc.tile_pool(name="esb", bufs=4) as esbp,
            tc.tile_pool(name="ysb", bufs=3) as ysbp,
            tc.tile_pool(name="wos", bufs=2) as wos,
            tc.tile_pool(name="acc", bufs=2, space="PSUM") as accp,
            tc.tile_pool(name="eps", bufs=3, space="PSUM") as epsp,
        ):
            xt = const.tile([P, CE, R], BF16, tag="xt")
            qt = const.tile([P, H, R], BF16, tag="qt")
            gt = const.tile([P, H, R], BF16, tag="gt")
            og = const.tile([P, H, R], BF16, tag="og")
            dz = const.tile([P, R], BF16, tag="dz")

            # ---- PE warm-up: matmuls with no DMA dependency, so the PE is
            # busy (and HAM-ramped) before the first x bytes land.
            nc.gpsimd.memset(dz[:], 0.0)
            for _ in range(N_DUMMY):
                dps = epsp.tile([P, 2, R], F32, tag="eps", name="dps")
                nc.tensor.matmul(dps[:, 0, :], lhsT=dz[:, :P], rhs=dz[:],
                                 start=True, stop=True)

            # ---- Startup DMAs in bandwidth order.
            wq_t = {}
            wg_t = {}
            wq_t[0] = wqp.tile([P, CE, P], BF16, tag="wq", name="wq0")

            nc.sync.dma_start(xt[:, 0:4, :], xt_d[:, 0:4, :])
            nc.sync.dma_start(wq_t[0][:], wqt_d[0])
            nc.sync.dma_start(xt[:, 4:8, :], xt_d[:, 4:8, :])
            nc.sync.dma_start(xt[:, 8:12, :], xt_d[:, 8:12, :])
            nc.sync.dma_start(xt[:, 12:14, :], xt_d[:, 12:14, :])
            nc.sync.dma_start(xt[:, 14:16, :], xt_d[:, 14:16, :])

            kt0 = kv.tile([P, MI, P], BF16, tag="kt")
            v0 = kv.tile([P, MI, P], BF16, tag="v")
            nc.sync.dma_start(kt0[:, 0:KV_SPLIT, :], kt_d[0][:, 0:KV_SPLIT, :])
            nc.sync.dma_start(v0[:, 0:KV_SPLIT, :], v_d[0][:, 0:KV_SPLIT, :])
            wq_t[1] = wqp.tile([P, CE, P], BF16, tag="wq", name="wq1")
            nc.sync.dma_start(wq_t[1][:], wqt_d[1])
            nc.sync.dma_start(kt0[:, KV_SPLIT:, :], kt_d[0][:, KV_SPLIT:, :])
            nc.sync.dma_start(v0[:, KV_SPLIT:, :], v_d[0][:, KV_SPLIT:, :])
            wg_t[0] = wgp.tile([P, CE, P], BF16, tag="wg", name="wg0")
            nc.sync.dma_start(wg_t[0][:], wgt_d[0])

            # ---- emitters ----
            def emit_q(h):
                if h not in wq_t:
                    wq_t[h] = wqp.tile([P, CE, P], BF16, tag="wq", name="wqh")
                    nc.sync.dma_start(wq_t[h][:], wqt_d[h])
                ps = accp.tile([P, R], F32, tag="acc", name="qps")
                for c in range(CE):
                    nc.tensor.matmul(
                        ps[:], lhsT=wq_t[h][:, c, :], rhs=xt[:, c, :],
                        start=(c == 0), stop=(c == CE - 1),
                    )
                nc.vector.tensor_copy(qt[:, h, :], ps[:])

            def emit_gate(j):
                if j not in wg_t:
                    wg_t[j] = wgp.tile([P, CE, P], BF16, tag="wg", name="wgh")
                    nc.sync.dma_start(wg_t[j][:], wgt_d[j])
                ps = accp.tile([P, R], F32, tag="acc", name="gps")
                for c in range(CE):
                    nc.tensor.matmul(
                        ps[:], lhsT=wg_t[j][:, c, :], rhs=xt[:, c, :],
                        start=(c == 0), stop=(c == CE - 1),
                    )
                nc.scalar.activation(
                    gt[:, j, :], ps[:], mybir.ActivationFunctionType.Silu
                )

            wo_tiles = {}

            def fetch_wo(n):
                wo_tiles[n] = wos.tile([P, CE, 512], BF16, tag="wo", name="wo")
                nc.sync.dma_start(wo_tiles[n][:], wout_d[n])

            def outproj_ops(n, t, col0=0, ncols=512, eps_psum=False):
                """Closure list: one matmul chunk each; last adds copy+DMA."""
                st = {}

                def op(ci):
                    if ci == 0:
                        if eps_psum:
                            full = epsp.tile([P, 2, R], F32, tag="eps", name="oeps")
                            st["ps"] = full[:, 0, 0:ncols]
                        else:
                            st["ps"] = accp.tile([P, ncols], F32, tag="acc",
                                                 name="ops_ps")
                    nc.tensor.matmul(
                        st["ps"][:],
                        lhsT=og[:, ci, t * P:(t + 1) * P],
                        rhs=wo_tiles[n][:, ci, col0:col0 + ncols],
                        start=(ci == 0), stop=(ci == CE - 1),
                    )
                    if ci == CE - 1:
                        ysb = ysbp.tile([P, ncols], F32, tag="ysb")
                        nc.vector.tensor_copy(ysb[:], st["ps"][:])
                        nc.sync.dma_start(
                            out_d[t * P:(t + 1) * P,
                                  n * 512 + col0: n * 512 + col0 + ncols],
                            ysb[:],
                        )

                return [lambda c=c: op(c) for c in range(CE)]

            def emit_attn(h, kt_t, v_t, fillers=None, ops_on_eps=False):
                fillers = fillers or {}
                if ops_on_eps:
                    # Head 0 only: keep the AV accumulator out of the acc
                    # ring so AV(0) does not wait on Q1 finishing through
                    # buffer reuse (head 0 is DMA/ACT paced, not PE paced).
                    full = epsp.tile([P, 2, R], F32, tag="eps", name="avps0")
                    ops_ps = full[:, 0, :]
                else:
                    ops_ps = accp.tile([P, R], F32, tag="acc", name="avps")
                for gi in range(NGROUPS):
                    i = 2 * gi
                    eps = epsp.tile([P, 2, R], F32, tag="eps", name="eps")
                    for k in range(2):
                        nc.tensor.matmul(
                            eps[:, k, :], lhsT=kt_t[:, i + k, :], rhs=qt[:, h, :],
                            start=True, stop=True,
                        )
                    esb = esbp.tile([P, 2, R], BF16, tag="esb")
                    nc.scalar.activation(
                        esb[:], eps[:], mybir.ActivationFunctionType.Silu
                    )
                    for k in range(2):
                        nc.tensor.matmul(
                            ops_ps[:], lhsT=v_t[:, i + k, :], rhs=esb[:, k, :],
                            start=(i + k == 0), stop=(i + k == MI - 1),
                        )
                    for f in fillers.get(gi, []):
                        f()
                return ops_ps

            # ---- main pipeline. The Tile scheduler interleaves the Q/gate
            # blocks into the silu-paced attn groups; gate(h) is emitted
            # AFTER attn(h) so its silu sits behind head h's energy silus in
            # the in-order ACT queue (ahead of them it would stall the head).
            # Q(h) blocks are emitted after the head that precedes their
            # consumer by one, so they never outrank attention energies.
            emit_q(0)
            for h in range(H):
                if h == 0:
                    kt_t, v_t = kt0, v0
                else:
                    kt_t = kv.tile([P, MI, P], BF16, tag="kt")
                    nc.sync.dma_start(kt_t[:], kt_d[h])
                    v_t = kv.tile([P, MI, P], BF16, tag="v")
                    nc.sync.dma_start(v_t[:], v_d[h])
                ops_ps = emit_attn(h, kt_t, v_t, ops_on_eps=(h == 0))
                if h == 0:
                    # Q1/Q2 here: after attn0 (so they cannot preempt the
                    # head-0 energy stream) and before gate0 (so their acc
                    # slots pair with Q-copies, not the late gate silu).
                    emit_q(1)
                    emit_q(2)
                emit_gate(h)
                nc.vector.tensor_mul(og[:, h, :], ops_ps[:], gt[:, h, :])
                if 1 <= h < H - 2:
                    emit_q(h + 2)

            # ---- Output projection ----
            # (0,0) and (0,1) on eps-pool psums: their chunks are free of
            # acc-ring coupling to og-mul(15), so the scheduler hoists them
            # into head 15's slack and across the og[15] silu->mul chain.
            fetch_wo(0)
            for t in range(4):
                for f in outproj_ops(0, t, eps_psum=(t < 2)):
                    f()
            for n in range(1, 4):
                fetch_wo(n)
                for t in range(4):
                    if n == 3 and t == 3:
                        # Final tile in two column halves: the first half's
                        # copy+DMA overlap the second half's matmuls.
                        for half in range(2):
                            for f in outproj_ops(n, t, col0=half * 256, ncols=256):
                                f()
                    else:
                        for f in outproj_ops(n, t):
                            f()

    nc.compile()
    return nc


def prep_inputs(x, Wq, k_weight, v_weight, Wg, Wout):
    """Host-side: shard x, pre-transpose + bf16-cast all operands."""
    bf = ml_dtypes.bfloat16
    xf = np.ascontiguousarray(np.asarray(x, dtype=np.float32).reshape(NCORES * R, E))

    wqt = np.ascontiguousarray(
        np.asarray(Wq, np.float32).T.reshape(CE, P, H, P).transpose(2, 1, 0, 3)
    ).astype(bf)
    wgt = np.ascontiguousarray(
        np.asarray(Wg, np.float32).T.reshape(CE, P, H, P).transpose(2, 1, 0, 3)
    ).astype(bf)
    kt = np.ascontiguousarray(
        (np.asarray(k_weight, np.float32) * SCALE).T.reshape(H, P, MI, P)
    ).astype(bf)
    v = np.ascontiguousarray(
        (np.asarray(v_weight, np.float32) * SCALE).reshape(MI, P, H, P).transpose(2, 1, 0, 3)
    ).astype(bf)
    wout = np.ascontiguousarray(
        np.asarray(Wout, np.float32).T.reshape(CE, P, 4, 512).transpose(2, 1, 0, 3)
    ).astype(bf)

    in_maps = []
    for c in range(NCORES):
        shard = xf[c * R:(c + 1) * R]  # [512, 2048]
        xt = np.ascontiguousarray(shard.T.reshape(CE, P, R).transpose(1, 0, 2)).astype(bf)
        in_maps.append(
            {"xt": xt, "wqt": wqt, "wgt": wgt, "kt": kt, "v": v, "woutt": wout}
        )
    return in_maps


_NC_CACHE = None


def get_nc():
    global _NC_CACHE
    if _NC_CACHE is None:
        _NC_CACHE = build_nc()
    return _NC_CACHE


def run(in_maps, trace=False):
    if trace:
        install_ntff_hook()
    return run_bass_kernel_spmd(
        get_nc(), in_maps, core_ids=list(range(NCORES)), trace=trace
    )


def kernel(x, Wq, k_weight, v_weight, Wg, Wout):
    B, N, Ein = x.shape
    in_maps = prep_inputs(x, Wq, k_weight, v_weight, Wg, Wout)
    res = run(in_maps, trace=False)
    out = np.concatenate([res.results[i]["out"] for i in range(NCORES)], axis=0)
    return out.reshape(B, N, Ein).astype(np.float32)


# revision 18
# speedup vs baseline: 1.0148x; 1.0148x over previous
"""Trainium2 Bass kernel for a 16-head linear-attention ("ALU") transformer block.

Reference computation (per row r of x, flattened over [B, N]):
    q  = x @ Wq.T                     # [R, 2048] -> 16 heads x 128
    g  = silu(x @ Wg.T)               # [R, 2048]
    e_h = silu(q_h @ (s*k_h).T)       # [R, 4096]   s = sqrt(128)
    o_h = e_h @ (s*v_h)               # [R, 128]
    out = (concat_h(o_h) * g) @ Wout.T

Strategy: pure data-parallel over the 4096 rows (512 rows/core, 8 cores, no
collectives). Weights are replicated; all operands are pre-transposed and cast
to bf16 on the host so every DMA is contiguous and every matmul contracts on
the partition axis. fp32 accumulation in PSUM throughout.
"""

import sys
import types

import numpy as np
import ml_dtypes

import concourse.bass as bass
import concourse.mybir as mybir
import concourse.tile as tile
from concourse import bacc
from concourse.bass_utils import run_bass_kernel_spmd

BF16 = mybir.dt.bfloat16
F32 = mybir.dt.float32

P = 128          # partitions / head dim
H = 16           # heads
CE = 16          # contraction chunks over E=2048
R = 512          # rows per core
MI = 32          # m-chunks per head (M=4096)
NCORES = 8
E = 2048
M = 4096
SCALE = float(np.float32(P) ** 0.5)

# Attention-phase energy groups: m-chunks per silu group (PSUM-bank limited:
# 2x3 banks energy double-buffer + 2x1 bank accumulators = 8 banks).
EGROUPS = [2] * 16
assert sum(EGROUPS) == MI


def install_ntff_hook():
    """Install the axon NTFF profiling hook that the stub `antenv` lacks."""
    import antenv

    if "antenv.axon_hooks" in sys.modules:
        return
    try:
        from trn_agent_boot.trn_boot import _ntff_profile_via_ctypes

        hook = _ntff_profile_via_ctypes("/opt/axon/libaxon_pjrt.so")
    except Exception:
        hook = None
    mod = types.ModuleType("antenv.axon_hooks")
    mod.get_axon_ntff_profile_hook = lambda: hook
    mod.set_axon_ntff_profile_hook = lambda h: None
    sys.modules["antenv.axon_hooks"] = mod
    antenv.axon_hooks = mod


def build_nc():
    nc = bacc.Bacc("TRN2", target_bir_lowering=False, debug=False, num_devices=NCORES)

    xt_d = nc.dram_tensor("xt", [P, CE, R], BF16, kind="ExternalInput").ap()
    wqt_d = nc.dram_tensor("wqt", [H, P, CE, P], BF16, kind="ExternalInput").ap()
    wgt_d = nc.dram_tensor("wgt", [H, P, CE, P], BF16, kind="ExternalInput").ap()
    kt_d = nc.dram_tensor("kt", [H, P, MI, P], BF16, kind="ExternalInput").ap()
    v_d = nc.dram_tensor("v", [H, P, MI, P], BF16, kind="ExternalInput").ap()
    wout_d = nc.dram_tensor("woutt", [4, P, CE, 512], BF16, kind="ExternalInput").ap()
    out_d = nc.dram_tensor("out", [R, E], F32, kind="ExternalOutput").ap()

    with tile.TileContext(nc) as tc:
        with (
            tc.tile_pool(name="const", bufs=1) as const,
            tc.tile_pool(name="wstream", bufs=2) as wstream,
            tc.tile_pool(name="kv", bufs=2) as kv,
            tc.tile_pool(name="esb", bufs=4) as esbp,
            tc.tile_pool(name="ysb", bufs=3) as ysbp,
            tc.tile_pool(name="wos", bufs=2) as wos,
            tc.tile_pool(name="acc", bufs=2, space="PSUM") as accp,
            tc.tile_pool(name="eps", bufs=3, space="PSUM") as epsp,
        ):
            xt = const.tile([P, CE, R], BF16, tag="xt")
            qt = const.tile([P, H, R], BF16, tag="qt")
            gt = const.tile([P, H, R], BF16, tag="gt")
            og = const.tile([P, H, R], BF16, tag="og")
            dz = const.tile([P, R], BF16, tag="dz")

            # PE warm-up: matmuls with no DMA dependency keep the PE busy
            # (and its HAM throughput ramp warm) until the first x bytes
            # land, so the real matmuls start at full rate.
            nc.gpsimd.memset(dz[:], 0.0)
            for _ in range(10):
                dps = epsp.tile([P, 2, R], F32, tag="eps", name="dps")
                nc.tensor.matmul(dps[:, 0, :], lhsT=dz[:, :P], rhs=dz[:],
                                 start=True, stop=True)

            # Prefetch head-0 Wq first, then x in 4 slices so the first Q
            # matmuls start as soon as the first slice lands.
            wq_first = wstream.tile([P, CE, P], BF16, tag="wq")
            nc.sync.dma_start(wq_first[:], wqt_d[0])
            for s in range(4):
                nc.sync.dma_start(xt[:, s * 4:(s + 1) * 4, :], xt_d[:, s * 4:(s + 1) * 4, :])

            def emit_q(h, wq_t=None):
                if wq_t is None:
                    wq_t = wstream.tile([P, CE, P], BF16, tag="wq")
                    nc.sync.dma_start(wq_t[:], wqt_d[h])
                ps = accp.tile([P, R], F32, tag="acc")
                for c in range(CE):
                    nc.tensor.matmul(
                        ps[:], lhsT=wq_t[:, c, :], rhs=xt[:, c, :],
                        start=(c == 0), stop=(c == CE - 1),
                    )
                nc.vector.tensor_copy(qt[:, h, :], ps[:])

            def emit_gate(j):
                wg_t = wstream.tile([P, CE, P], BF16, tag="wg")
                nc.sync.dma_start(wg_t[:], wgt_d[j])
                ps = accp.tile([P, R], F32, tag="acc")
                for c in range(CE):
                    nc.tensor.matmul(
                        ps[:], lhsT=wg_t[:, c, :], rhs=xt[:, c, :],
                        start=(c == 0), stop=(c == CE - 1),
                    )
                nc.scalar.activation(gt[:, j, :], ps[:], mybir.ActivationFunctionType.Silu)

            def emit_attn(h):
                kt_t = kv.tile([P, MI, P], BF16, tag="kt")
                nc.sync.dma_start(kt_t[:], kt_d[h])
                v_t = kv.tile([P, MI, P], BF16, tag="v")
                nc.sync.dma_start(v_t[:], v_d[h])

                ops = accp.tile([P, R], F32, tag="acc")
                i = 0
                for g in EGROUPS:
                    eps = epsp.tile([P, 2, R], F32, tag="eps")
                    for gi in range(g):
                        nc.tensor.matmul(
                            eps[:, gi, :], lhsT=kt_t[:, i + gi, :], rhs=qt[:, h, :],
                            start=True, stop=True,
                        )
                    esb = esbp.tile([P, 2, R], BF16, tag="esb")
                    nc.scalar.activation(
                        esb[:, :g, :], eps[:, :g, :], mybir.ActivationFunctionType.Silu
                    )
                    for gi in range(g):
                        nc.tensor.matmul(
                            ops[:], lhsT=v_t[:, i + gi, :], rhs=esb[:, gi, :],
                            start=(i + gi == 0), stop=(i + gi == MI - 1),
                        )
                    i += g
                nc.vector.tensor_mul(og[:, h, :], ops[:], gt[:, h, :])

            # Software pipeline: attention of head h is ACT(silu)-paced, so the
            # independent Q(h+2)/gate(h) matmul groups are emitted between heads
            # for the scheduler to fill TensorE gaps with.
            emit_q(0, wq_t=wq_first)
            emit_q(1)
            emit_gate(0)
            for h in range(H):
                emit_attn(h)
                if h + 2 < H:
                    emit_q(h + 2)
                if h + 1 < H:
                    emit_gate(h + 1)

            # ---- Output projection: out = (og)^T @ WoutT ----
            for n in range(4):
                wo_t = wos.tile([P, CE, 512], BF16, tag="wo")
                nc.sync.dma_start(wo_t[:], wout_d[n])
                for t in range(4):
                    last = (n == 3 and t == 3)
                    if not last:
                        ps = accp.tile([P, R], F32, tag="acc")
                        for cv in range(CE):
                            nc.tensor.matmul(
                                ps[:], lhsT=og[:, cv, t * P:(t + 1) * P], rhs=wo_t[:, cv, :],
                                start=(cv == 0), stop=(cv == CE - 1),
                            )
                        ysb = ysbp.tile([P, 512], F32, tag="ysb")
                        nc.vector.tensor_copy(ysb[:], ps[:])
                        nc.sync.dma_start(
                            out_d[t * P:(t + 1) * P, n * 512:(n + 1) * 512], ysb[:]
                        )
                    else:
                        # Final tile: two column halves so the first half's
                        # copy+DMA overlap the second half's matmuls, keeping
                        # the post-last-matmul serial chain short.
                        for half in range(2):
                            ps = accp.tile([P, R // 2], F32, tag="acc")
                            for cv in range(CE):
                                nc.tensor.matmul(
                                    ps[:],
                                    lhsT=og[:, cv, t * P:(t + 1) * P],
                                    rhs=wo_t[:, cv, half * 256:(half + 1) * 256],
                                    start=(cv == 0), stop=(cv == CE - 1),
                                )
                            ysb = ysbp.tile([P, R // 2], F32, tag="ysb")
                            nc.vector.tensor_copy(ysb[:], ps[:])
                            nc.sync.dma_start(
                                out_d[t * P:(t + 1) * P,
                                      n * 512 + half * 256: n * 512 + (half + 1) * 256],
                                ysb[:],
                            )

    nc.compile()
    return nc


def prep_inputs(x, Wq, k_weight, v_weight, Wg, Wout):
    """Host-side: shard x, pre-transpose + bf16-cast all operands."""
    bf = ml_dtypes.bfloat16
    xf = np.ascontiguousarray(np.asarray(x, dtype=np.float32).reshape(NCORES * R, E))

    wqt = np.ascontiguousarray(
        np.asarray(Wq, np.float32).T.reshape(CE, P, H, P).transpose(2, 1, 0, 3)
    ).astype(bf)
    wgt = np.ascontiguousarray(
        np.asarray(Wg, np.float32).T.reshape(CE, P, H, P).transpose(2, 1, 0, 3)
    ).astype(bf)
    kt = np.ascontiguousarray(
        (np.asarray(k_weight, np.float32) * SCALE).T.reshape(H, P, MI, P)
    ).astype(bf)
    v = np.ascontiguousarray(
        (np.asarray(v_weight, np.float32) * SCALE).reshape(MI, P, H, P).transpose(2, 1, 0, 3)
    ).astype(bf)
    wout = np.ascontiguousarray(
        np.asarray(Wout, np.float32).T.reshape(CE, P, 4, 512).transpose(2, 1, 0, 3)
    ).astype(bf)

    in_maps = []
    for c in range(NCORES):
        shard = xf[c * R:(c + 1) * R]  # [512, 2048]
        xt = np.ascontiguousarray(shard.T.reshape(CE, P, R).transpose(1, 0, 2)).astype(bf)
        in_maps.append(
            {"xt": xt, "wqt": wqt, "wgt": wgt, "kt": kt, "v": v, "woutt": wout}
        )
    return in_maps


_NC_CACHE = None


def get_nc():
    global _NC_CACHE
    if _NC_CACHE is None:
        _NC_CACHE = build_nc()
    return _NC_CACHE


def run(in_maps, trace=False):
    if trace:
        install_ntff_hook()
    return run_bass_kernel_spmd(
        get_nc(), in_maps, core_ids=list(range(NCORES)), trace=trace
    )


def kernel(x, Wq, k_weight, v_weight, Wg, Wout):
    B, N, Ein = x.shape
    in_maps = prep_inputs(x, Wq, k_weight, v_weight, Wg, Wout)
    res = run(in_maps, trace=False)
    out = np.concatenate([res.results[i]["out"] for i in range(NCORES)], axis=0)
    return out.reshape(B, N, Ein).astype(np.float32)

